# revision 6
# baseline (speedup 1.0000x reference)
"""Trainium2 Bass kernel for nn_AttentionTypeConcatSheafLearner.

Math restructuring (exact, no approximation):
  reference per edge e=(r,c):
    h   = [x[r], x[c], nt_oh[r], nt_oh[c], et_oh[e]]          (288)
    z   = relu(LN(h; ln_w, ln_b) @ W1 + b1)                   (64)
    o   = z @ W2 + b2                                         (16)
    out = I4 - softmax(o.reshape(4,4), axis=-1)

  LN(h) @ W1 = rstd * (h @ W1' - mu * s) + (b1 + ln_b @ W1)
    with W1' = diag(ln_w) @ W1, s = colsum(W1'); mu/rstd per-edge scalars.
  h @ W1' = x[r]@A + x[c]@B + C1[nt[r]] + C2[nt[c]] + Cet[et]
  sum(h) = sx[r]+sx[c]+3, sum(h^2) = sqx[r]+sqx[c]+3  (one-hot parts).

  Per-node tables (host, O(N)):
    u~[n] = x[n]@A + C1[nt[n]] - (sx[n]/288)*s | sx[n] | sqx[n]   (66 f32)
    v~[n] = x[n]@B + C2[nt[n]] - (sx[n]/288)*s | sx[n] | sqx[n]   (66 f32)

  Edges are SORTED BY TYPE on the host and sharded 2 types per core, each
  type padded to exactly TMACRO macro-tiles, so each macro-tile is
  type-pure and the Cet[et] term folds into a per-macro bias constant:
    bias_t = b0 + Cet[t]*... -> b0t[t] = (b1 + ln_b@W1) ... applied after
  rstd scaling, while cet~[t] = Cet[t] - (3/288)*s enters before scaling.
  Per edge (tile-level, edge-major: 128 edges on partitions):
    g  = u~[r] + v~[c]                  (2 indirect gathers + 1 DVE add)
    S1 = g[64], S2 = g[65]
    mu = (S1+3)/288 ; var = (S2+3)/288 + eps - mu^2 ; rstd = 1/sqrt(var)
    a  = relu(rstd * (g[0:64] + cet~[t]) + b0)
       = relu(rstd * g[0:64] + (rstd*cet~[t] + b0))   <- NO: rstd per edge.
    So cet~[t] must be added BEFORE the rstd multiply:
    a  = relu(rstd * gc + b0), gc = g[0:64] + cet~[t]  (t fixed per macro:
         the add is fused into the rstd-multiply's second operand via a
         per-macro broadcast row -> one extra DVE add batched per macro)
    o  = [a | 1] @ [W2 ; b2];  out = I4 - rowsoftmax4(o)
"""

import os
import numpy as np

N, E = 50000, 800000
C, NT, ET, H, D = 128, 8, 16, 64, 4
TOTAL_IN = 2 * C + 2 * NT + ET  # 288
EPS = 1e-5

P = 128
G = 16                      # groups per macro tile
EDGES_PER_MACRO = P * G     # 2048
NCORES = 8
TYPES_PER_CORE = ET // NCORES   # 2
TMACRO = 25                 # macros per type bucket (25*2048=51200 >= any count)
NMACRO = TYPES_PER_CORE * TMACRO  # 50
E_TYPE_PAD = TMACRO * EDGES_PER_MACRO   # 51200
E_PAD = NMACRO * EDGES_PER_MACRO        # 102400
NGROUPS = NMACRO * G        # 800
ROW_W = 66                  # u~ | sx | sqx
AW = 65                     # a | ones

_CACHE = {}
LAST_RESULTS = None  # BassKernelResults stash for test harness


def _build_program():
    import concourse.bacc as bacc
    import concourse.bass as bass
    import concourse.tile as tile
    import concourse.mybir as mybir
    from concourse.masks import make_identity

    f32 = mybir.dt.float32
    i32 = mybir.dt.int32
    Alu = mybir.AluOpType
    Act = mybir.ActivationFunctionType

    nc = bacc.Bacc("TRN2", target_bir_lowering=False, debug=False,
                   num_devices=NCORES)

    u_tab = nc.dram_tensor("u_tab", [N, ROW_W], f32, kind="ExternalInput").ap()
    v_tab = nc.dram_tensor("v_tab", [N, ROW_W], f32, kind="ExternalInput").ap()
    row2d = nc.dram_tensor("row2d", [P, NGROUPS], i32, kind="ExternalInput").ap()
    col2d = nc.dram_tensor("col2d", [P, NGROUPS], i32, kind="ExternalInput").ap()
    # per-macro-bucket rows: cet~ (pre-scale add) and b0 (post-scale add),
    # replicated across partitions: [P, TYPES_PER_CORE*64] each
    cetrow = nc.dram_tensor("cetrow", [P, TYPES_PER_CORE * 64], f32,
                            kind="ExternalInput").ap()
    b0row = nc.dram_tensor("b0row", [P, 64], f32, kind="ExternalInput").ap()
    w2a = nc.dram_tensor("w2a", [AW, 16], f32, kind="ExternalInput").ap()
    irow = nc.dram_tensor("irow", [P, 16], f32, kind="ExternalInput").ap()
    out_d = nc.dram_tensor("out", [NMACRO, P, G * 16], f32,
                           kind="ExternalOutput").ap()

    with tile.TileContext(nc) as tc:
        with (
            tc.tile_pool(name="const", bufs=1) as constp,
            tc.tile_pool(name="gmac", bufs=3) as gpool,
            tc.tile_pool(name="amac", bufs=2) as apool,
            tc.tile_pool(name="atr", bufs=4) as atp,
            tc.tile_pool(name="stats", bufs=2) as stp,
            tc.tile_pool(name="expt", bufs=2) as expp,
            tc.tile_pool(name="outt", bufs=2) as outp,
            tc.tile_pool(name="pstr", bufs=4, space="PSUM") as ps_t,
            tc.tile_pool(name="pso", bufs=2, space="PSUM") as ps_o,
        ):
            # ---- persistent constants ----
            idx_r = constp.tile([P, NGROUPS], i32)
            idx_c = constp.tile([P, NGROUPS], i32)
            nc.sync.dma_start(idx_r[:], row2d)
            nc.sync.dma_start(idx_c[:], col2d)
            w2a_t = constp.tile([AW, 16], f32)
            nc.sync.dma_start(w2a_t[:], w2a)
            cet_t = constp.tile([P, TYPES_PER_CORE * 64], f32)
            nc.sync.dma_start(cet_t[:], cetrow)
            b0_t = constp.tile([P, 64], f32)
            nc.sync.dma_start(b0_t[:], b0row)
            irow_t = constp.tile([P, 16], f32)
            nc.sync.dma_start(irow_t[:], irow)
            id_t = constp.tile([P, P], f32)
            make_identity(nc, id_t[:])

            def mid_bc(ap2, n):
                # [P, w] -> [P, n(bcast), w]
                (ps, pc), (fs, fc) = ap2.ap
                return bass.AP(ap2.tensor, ap2.offset,
                               [[ps, pc], [0, n], [fs, fc]])

            def bc(ap2, n):
                return bass.AP(ap2.tensor, ap2.offset,
                               list(ap2.ap) + [[0, n]])

            b0_bc3 = mid_bc(b0_t[:], G)
            irow_bc3 = mid_bc(irow_t[:], G)

            for m in range(NMACRO):
                tloc = m // TMACRO
                cet_bc3 = mid_bc(cet_t[:, tloc * 64:(tloc + 1) * 64], G)
                # ---- gathers: gu = u~[r], gv = v~[c]; g = gu + gv ----
                gu = gpool.tile([P, G * ROW_W], f32, tag="gu")
                gv = gpool.tile([P, G * ROW_W], f32, tag="gv")
                gu3 = gu[:].rearrange("p (g w) -> p g w", w=ROW_W)
                gv3 = gv[:].rearrange("p (g w) -> p g w", w=ROW_W)
                for gi in range(G):
                    col = m * G + gi
                    nc.gpsimd.indirect_dma_start(
                        out=gu3[:, gi, :], out_offset=None, in_=u_tab,
                        in_offset=bass.IndirectOffsetOnAxis(
                            ap=idx_r[:, col:col + 1], axis=0))
                    nc.gpsimd.indirect_dma_start(
                        out=gv3[:, gi, :], out_offset=None, in_=v_tab,
                        in_offset=bass.IndirectOffsetOnAxis(
                            ap=idx_c[:, col:col + 1], axis=0))
                nc.vector.tensor_tensor(gu[:], gu[:], gv[:], Alu.add)

                # ---- edge scalars, batched over G groups: [P, G] ops ----
                st = stp.tile([P, 6 * G], f32)
                s_m, s_q, s_mm, s_var, s_ir, s_rstd = (
                    st[:, k * G:(k + 1) * G] for k in range(6))
                S1 = gu[:, 64::ROW_W]
                S2 = gu[:, 65::ROW_W]
                nc.vector.tensor_scalar(s_m, S1, 1.0 / TOTAL_IN,
                                        3.0 / TOTAL_IN, Alu.mult, Alu.add)
                nc.vector.tensor_scalar(s_q, S2, 1.0 / TOTAL_IN,
                                        3.0 / TOTAL_IN + EPS, Alu.mult, Alu.add)
                nc.vector.tensor_tensor(s_mm, s_m, s_m, Alu.mult)
                nc.vector.tensor_tensor(s_var, s_q, s_mm, Alu.subtract)
                nc.scalar.sqrt(s_ir, s_var)
                nc.vector.reciprocal(s_rstd, s_ir)

                # ---- a = relu(rstd * (g64 + cet~[t]) + b0), ones col ----
                a = apool.tile([P, G * AW], f32)
                a3 = a[:].rearrange("p (g w) -> p g w", w=AW)
                av = a3[:, :, 0:64]
                nc.vector.tensor_tensor(av, gu3[:, :, 0:64], cet_bc3, Alu.add)
                nc.vector.tensor_tensor(av, av, bc(s_rstd, 64), Alu.mult)
                nc.vector.tensor_tensor(av, av, b0_bc3, Alu.add)
                nc.vector.memset(a3[:, :, 64], 1.0)
                nc.scalar.activation(av, av, Act.Relu)

                # ---- per group: PE transpose, copy, W2 matmul ----
                ops = ps_o.tile([P, G * 16], f32)
                for gi in range(G):
                    at_ps = ps_t.tile([AW, P], f32)
                    nc.tensor.transpose(at_ps[:], a3[:, gi, :], id_t[:])
                    at_sb = atp.tile([AW, P], f32)
                    nc.scalar.copy(at_sb[:], at_ps[:])
                    nc.tensor.matmul(ops[:, gi * 16:(gi + 1) * 16],
                                     lhsT=at_sb[:], rhs=w2a_t[:],
                                     start=True, stop=True)

                # ---- batched softmax tail: out = I4 - exp/rowsum ----
                ex = expp.tile([P, G * 16], f32)
                nc.scalar.activation(ex[:], ops[:], Act.Exp)
                ex3 = ex[:].rearrange("p (r w) -> p r w", w=4)
                sums = stp.tile([P, 4 * G], f32)
                nc.vector.tensor_reduce(sums[:], ex3, mybir.AxisListType.X,
                                        Alu.add)
                rec = stp.tile([P, 4 * G], f32)
                nc.vector.reciprocal(rec[:], sums[:])
                ot = outp.tile([P, G * 16], f32)
                ot3 = ot[:].rearrange("p (r w) -> p r w", w=4)
                nc.vector.tensor_tensor(ot3, ex3, bc(rec[:], 4), Alu.mult)
                otg = ot[:].rearrange("p (g w) -> p g w", w=16)
                nc.vector.tensor_tensor(otg, irow_bc3, otg, Alu.subtract)
                nc.sync.dma_start(out_d[m], ot[:])

    nc.compile()
    return nc


def _prep_host(x, edge_index, edge_types, node_types, ln_w, ln_b, W1, b1, W2, b2):
    x = np.asarray(x, np.float32)
    ln_w = np.asarray(ln_w, np.float32)
    ln_b = np.asarray(ln_b, np.float32)
    W1 = np.asarray(W1, np.float32)
    b1 = np.asarray(b1, np.float32)
    W2 = np.asarray(W2, np.float32)
    b2 = np.asarray(b2, np.float32)

    W1p = ln_w[:, None] * W1                      # [288, 64]
    s = W1p.sum(0)                                # [64]
    b0 = b1 + ln_b @ W1                           # [64]
    A = W1p[0:C]
    B = W1p[C:2 * C]
    C1 = W1p[2 * C:2 * C + NT]
    C2 = W1p[2 * C + NT:2 * C + 2 * NT]
    Cet = W1p[2 * C + 2 * NT:]                    # [16, 64]
    cet_r = Cet - (3.0 / TOTAL_IN) * s[None, :]   # [16, 64] pre-scale add

    sx = x.sum(1)
    sqx = (x * x).sum(1)
    nt = np.asarray(node_types).astype(np.int64)
    mu_term = (sx / TOTAL_IN)[:, None] * s[None, :]
    u = x @ A + C1[nt] - mu_term
    v = x @ B + C2[nt] - mu_term
    u_tab = np.concatenate([u, sx[:, None], sqx[:, None]], 1).astype(np.float32)
    v_tab = np.concatenate([v, sx[:, None], sqx[:, None]], 1).astype(np.float32)

    w2a = np.concatenate([W2, b2[None, :]], 0).astype(np.float32)
    b0row = np.tile(b0[None, :].astype(np.float32), (P, 1))
    irow = np.tile(np.eye(D, dtype=np.float32).reshape(1, 16), (P, 1))

    row = np.asarray(edge_index[0]).astype(np.int32)
    col = np.asarray(edge_index[1]).astype(np.int32)
    et = np.asarray(edge_types).astype(np.int64)

    order = np.argsort(et, kind="stable")
    counts = np.bincount(et, minlength=ET)
    assert counts.max() <= E_TYPE_PAD, counts.max()
    starts = np.zeros(ET + 1, np.int64)
    np.cumsum(counts, out=starts[1:])

    in_maps = []
    unscatter = []   # per core: list of (type_global_positions, bucket_slot)
    for c in range(NCORES):
        seq = np.zeros(E_PAD, np.int64)   # global edge ids, padded
        un = []
        for k in range(TYPES_PER_CORE):
            t = c * TYPES_PER_CORE + k
            ids = order[starts[t]:starts[t + 1]]
            seq[k * E_TYPE_PAD:k * E_TYPE_PAD + len(ids)] = ids
            un.append((ids, k))
        un_core = un

        def shard2d(arr):
            sh = arr[seq].astype(np.int32)
            return np.ascontiguousarray(
                sh.reshape(NMACRO, P, G).transpose(1, 0, 2).reshape(P, NGROUPS))

        cetrow = np.tile(
            cet_r[c * TYPES_PER_CORE:(c + 1) * TYPES_PER_CORE].reshape(
                1, TYPES_PER_CORE * 64), (P, 1)).astype(np.float32)

        in_maps.append({
            "u_tab": u_tab, "v_tab": v_tab,
            "row2d": shard2d(row), "col2d": shard2d(col),
            "cetrow": cetrow, "b0row": b0row, "w2a": w2a, "irow": irow,
        })
        unscatter.append(un_core)
    return in_maps, unscatter


def kernel(**inputs) -> np.ndarray:
    global LAST_RESULTS
    from concourse.bass_utils import run_bass_kernel_spmd

    if "nc" not in _CACHE:
        _CACHE["nc"] = _build_program()
    nc = _CACHE["nc"]

    in_maps, unscatter = _prep_host(**{k: inputs[k] for k in
                                       ("x", "edge_index", "edge_types",
                                        "node_types", "ln_w", "ln_b", "W1",
                                        "b1", "W2", "b2")})

    res = run_bass_kernel_spmd(nc, in_maps, core_ids=list(range(NCORES)))
    LAST_RESULTS = res

    full = np.empty((E, 16), np.float32)
    for c in range(NCORES):
        rows = res.results[c]["out"].reshape(E_PAD, 16)
        for ids, k in unscatter[c]:
            full[ids] = rows[k * E_TYPE_PAD:k * E_TYPE_PAD + len(ids)]
    return full.reshape(E, D, D)
